# revision 41
# baseline (speedup 1.0000x reference)
# Trainium2 Bass kernel for nn_Attention_60464549593105.
#
# Math (per batch b, spatial point (h,w), seq s):
#   energy[k] = tanh( We @ enc[:,s] + Wh @ hidden + b_att )      (K=128)
#   score[s]  = W_v . energy
#   out[s]    = softmax_s(score)
#
# Strategy: shard the H axis across 8 cores (8 rows each) so softmax over
# seq is core-local (no collectives).
#
# Changes vs the f32/f32r baseline (237 us -> ~192 us measured):
#   - encoder_outputs / hidden / weights are cast to bf16 on the HOST, so
#     the 64 MiB/core HBM stream halves to 32 MiB. Measured quantization
#     error: 2.1e-3 global rel (vs the 2e-2 gate). bf16 also enables FWL
#     (fast weight load), which f32r blocked (LDWEIGHTS 162 -> 96 ns).
#   - tanh batched over 2 seq positions: [K, 1024] psum tiles (2 banks,
#     3 bufs) halve the ACT per-instruction access penalty (~245 ns/op).
#   - scores accumulate in PARITY groups (even s -> psum partitions 0..31,
#     odd s -> 32..63 of one bank, via tile_position col groups), so
#     consecutive score LDWEIGHTS pull ahead in the PE reorder window.
#     Since both groups stay open at once and matmul start=True clears
#     has_written bits region-scoped, the bank is opened by one
#     zero-writing matmul (wvs cols 64..127 are zero) and all score
#     matmuls use start=False. Output DRAM is [B, S/2, 2, FREE], which
#     flattens back to s-order.
#   - two alternating score banks so batch b+1's scores start while batch
#     b's softmax tail drains; the sum matmul overwrites the consumed
#     score bank (no extra bank).
#   - softmax: one exp per batch (f32r, reused as sum-matmul rhs and
#     numerator), one 64-contraction sum matmul broadcast to 128 rows,
#     reciprocal_approx_fast on DVE, final mul on the idle gpsimd engine
#     (keeps the in-order DVE add queue from bubbling the tanh stream).
#   - proj_h add balanced between DVE (tensor_add) and PE (accumulating
#     whT matmul) via ADD_PE_EVERY; enc DMAs issue from the gpsimd queue,
#     consts/outputs from sync. Last-batch softmax pipelined over two
#     FREE-halves; final 1-seq chunks shorten the drain.
# Engine budget at nominal clock (per core): scalar(tanh) ~145 us is the
# wall, tensor ~142, vector ~127, DMA ~100; span ~195 incl. ~18 us fixed
# framework pre/postamble. Run-to-run clock throttling adds up to +20%.

import numpy as np
import ml_dtypes

B, D, E, S, H, W = 4, 128, 128, 64, 64, 64
K = 128
NCORES = 8
HSH = H // NCORES          # h rows per core
FREE = HSH * W             # free-dim elements per (b, s) tile
TCH = 2                    # seq positions per tanh chunk (psum tile = 2 banks)
SCH = 24                   # seq positions per enc DMA chunk (3 MiB bf16)
ENC_BUFS = 3
ADD_PE_EVERY = 5           # chunk % ADD_PE_EVERY == 0 -> proj_h via PE matmul

_CACHE = {}


def _build_bass():
    import concourse.bacc as bacc
    import concourse.mybir as mybir
    import concourse.tile as tile
    from contextlib import ExitStack

    f32 = mybir.dt.float32
    f32r = mybir.dt.float32r
    bf16 = mybir.dt.bfloat16
    AF = mybir.ActivationFunctionType

    nc = bacc.Bacc("TRN2", target_bir_lowering=False, debug=False)
    enc = nc.dram_tensor("enc", [B, E, S * FREE], bf16, kind="ExternalInput")
    hid = nc.dram_tensor("hid", [B, D, FREE], bf16, kind="ExternalInput")
    weT = nc.dram_tensor("weT", [E, K], bf16, kind="ExternalInput")
    whT = nc.dram_tensor("whT", [D, K], bf16, kind="ExternalInput")
    batt = nc.dram_tensor("batt", [K, 1], f32, kind="ExternalInput")
    wvs = nc.dram_tensor("wvs", [K, 2 * S], bf16, kind="ExternalInput")
    # out is declared [B, S/2, 2, FREE]: row r, parity p holds seq s=2r+p,
    # which flattens to plain [B, S, FREE] order. The score psum packs even
    # s at partitions 0..31 and odd s at 32..63 (parity groups) so that
    # consecutive score matmuls target different PE column groups and their
    # LDWEIGHTS can be pulled ahead by the PE reorder window.
    out = nc.dram_tensor("out", [B, S // 2, 2, FREE], f32, kind="ExternalOutput")

    SG = S // 2            # 32-row score groups (A: s 0..31, B: s 32..63)
    CPB = S // TCH         # tanh chunks per batch

    with tile.TileContext(nc) as tc, ExitStack() as ctx:
        consts = ctx.enter_context(tc.tile_pool(name="consts", bufs=1))
        encp = ctx.enter_context(tc.tile_pool(name="encp", bufs=ENC_BUFS))
        epsum = ctx.enter_context(tc.tile_pool(name="epsum", bufs=3, space="PSUM"))
        # two alternating score banks: rows 0..63 = parity-grouped score
        # accumulators, rows 64..127 = the softmax-sum broadcast (written
        # strictly after exp consumed the scores). Alternating banks lets
        # batch b+1's scores start while batch b's softmax tail still reads
        # its own bank.
        scpsum = ctx.enter_context(tc.tile_pool(name="scpsum", bufs=2, space="PSUM"))
        work = ctx.enter_context(tc.tile_pool(name="work", bufs=4))
        smax = ctx.enter_context(tc.tile_pool(name="smax", bufs=2))

        # ramp: weT + whT + hid[0] first so chunk-0 matmuls (pe_add path)
        # can start as early as possible; enc chunks go on the (otherwise
        # idle) gpsimd queue so their issue overlaps the sync-queue consts.
        weT_sb = consts.tile([E, K], bf16)
        nc.sync.dma_start(out=weT_sb, in_=weT[:])
        whT_sb = consts.tile([D, K], bf16)
        nc.sync.dma_start(out=whT_sb, in_=whT[:])
        hid_sb = consts.tile([D, B, FREE], bf16)
        nc.sync.dma_start(out=hid_sb[:, 0, :], in_=hid[0])
        batt_sb = consts.tile([K, 1], f32)
        nc.sync.dma_start(out=batt_sb, in_=batt[:])
        wvs_sb = consts.tile([K, 2 * S], bf16)
        nc.sync.dma_start(out=wvs_sb, in_=wvs[:])
        for b in range(1, B):
            nc.sync.dma_start(out=hid_sb[:, b, :], in_=hid[b])
        ones_tmp = consts.tile([S, 2 * S], f32)
        nc.vector.memset(ones_tmp, 1.0)
        ones_sb = consts.tile([S, 2 * S], f32r)
        nc.vector.tensor_copy(ones_sb, ones_tmp)

        # proj_h (repeated TCH times along free) per batch, f32 in SBUF
        projh2_sb = consts.tile([K, B, TCH * FREE], f32)

        def emit_projh(b):
            # matmul output is capped at one psum bank (512 f32) -> one
            # matmul per FREE-slice of the psum tile
            ph_ps = epsum.tile([K, TCH * FREE], f32, tag="e_ps", name="ph_ps")
            for jj in range(TCH):
                nc.tensor.matmul(ph_ps[:, jj * FREE : (jj + 1) * FREE],
                                 lhsT=whT_sb, rhs=hid_sb[:, b, :],
                                 start=True, stop=True)
            nc.vector.tensor_copy(projh2_sb[:, b, :], ph_ps)

        def softmax_tail(b, sc, expv, split=1, do_exp=False):
            # sum over s (64-deep contraction), broadcast to all 128 rows of
            # the score bank, overwriting it (safe: exp has already consumed
            # rows 0..63 before the sum matmul runs; matmul out starts at
            # partition 0 as HW requires).
            # split>1 pipelines exp/sum/recip/mul/DMA over FREE-slices to
            # shorten the end-of-kernel drain (only useful for the last b,
            # where exp is also emitted here instead of at batch end).
            rec = smax.tile([S, FREE], f32, tag="rec", name="rec")
            ob = smax.tile([S, FREE], f32, tag="ob", name="ob")
            fs = FREE // split
            if do_exp:
                # all exp slices first: the sum matmul overwrites the score
                # bank, so a later exp slice reading it would serialize
                # behind an earlier slice's sum (bank-aware tracking)
                for p in range(split):
                    sl = slice(p * fs, (p + 1) * fs)
                    nc.scalar.activation(expv[:, sl], sc[: S, sl], AF.Exp)
            for p in range(split):
                sl = slice(p * fs, (p + 1) * fs)
                nc.tensor.matmul(sc[:, sl], lhsT=ones_sb,
                                 rhs=expv[:, sl], start=True, stop=True,
                                 skip_group_check=True)
                nc.vector.reciprocal_approx_fast(out=rec[:, sl],
                                                 in_=sc[: S, sl])
                # the mul runs on the (otherwise idle) gpsimd engine so the
                # softmax tail doesn't delay the in-order DVE add queue
                # (which would bubble the tanh stream); all operands SBUF.
                nc.gpsimd.tensor_mul(out=ob[:, sl], in0=expv[:, sl],
                                     in1=rec[:, sl])
                for par in range(2):
                    nc.sync.dma_start(
                        out=out[b, :, par, sl],
                        in_=ob[par * SG : (par + 1) * SG, sl])

        # projh is emitted per batch, just-in-time: emitting all four up
        # front once stalled the whole ramp ~25us — the in-order PE queue
        # blocked at its head on hid[1..3] DMAs that were queued behind the
        # enc-chunk flood on the DMA engines.
        pending_tail = None
        for b in range(B):
            emit_projh(b)
            # score psum: one bank; even s accumulate at partitions 0..31,
            # odd s at 32..63 (parity groups -> alternating PE col_grps so
            # score LDWEIGHTS pull ahead). Both groups stay open at once, so
            # no score matmul may use start=True (start clears has_written
            # for regions beyond its own rows). Instead one matmul writes
            # zeros (wvs cols 64..127 are all zero) to all 64 rows with
            # start=True; the scores then accumulate onto it.
            sc = scpsum.tile([2 * S, FREE], f32, tag="sc", name="sc")
            nc.tensor.matmul(sc[: 2 * SG, :], lhsT=wvs_sb[:, S : 2 * S],
                             rhs=hid_sb[:, b, :],
                             start=True, stop=False, skip_group_check=True)
            expv = smax.tile([S, FREE], f32r, tag="expv", name="expv")
            if b == 0:        # ramp-up: compute starts after the first chunk
                chunks = [2, 2, 4, 8, 24, 24]
            elif b < B - 1:
                chunks = [24, 24, 16]
            else:             # tiny tail chunks shorten the final drain
                chunks = [24, 24, 14, 1, 1]
            off = 0
            ci = 0
            for csz in chunks:
                et = encp.tile([E, SCH * FREE], bf16, tag="et", name="et")
                nc.gpsimd.dma_start(
                    out=et[:, : csz * FREE],
                    in_=enc[b, :, off * FREE : (off + csz) * FREE],
                )
                for j in range(0, csz, TCH):
                    g = min(TCH, csz - j)
                    s0 = off + j
                    e_ps = epsum.tile([K, TCH * FREE], f32, tag="e_ps",
                                      name="e_ps")
                    # chunks 0-1 of each batch always use the PE add path:
                    # the DVE path would wait on the projh2 psum->sbuf copy,
                    # bubbling the tanh stream at every batch transition
                    pe_add = ci % ADD_PE_EVERY == 0 or ci == 1
                    for jj in range(g):
                        nc.tensor.matmul(
                            e_ps[:, jj * FREE : (jj + 1) * FREE],
                            lhsT=weT_sb,
                            rhs=et[:, (j + jj) * FREE : (j + jj + 1) * FREE],
                            start=True, stop=not pe_add)
                    if pe_add:
                        for jj in range(g):
                            nc.tensor.matmul(
                                e_ps[:, jj * FREE : (jj + 1) * FREE],
                                lhsT=whT_sb, rhs=hid_sb[:, b, :],
                                start=False, stop=True)
                    else:
                        nc.vector.tensor_add(
                            out=e_ps[:, : g * FREE],
                            in0=e_ps[:, : g * FREE],
                            in1=projh2_sb[:, b, : g * FREE])
                    th = work.tile([K, TCH * FREE], bf16, tag="th", name="th")
                    nc.scalar.activation(th[:, : g * FREE],
                                         e_ps[:, : g * FREE],
                                         AF.Tanh, bias=batt_sb)
                    for jj in range(g):
                        s = s0 + jj
                        r, grp = divmod(s, 2)   # parity groups
                        nc.tensor.matmul(
                            sc[grp * SG : (grp + 1) * SG, :],
                            lhsT=wvs_sb[:, (S - 1) - r : (S - 1) - r + SG],
                            rhs=th[:, jj * FREE : (jj + 1) * FREE],
                            start=False,
                            stop=(s == S - 1),
                            skip_group_check=True,
                        )
                    ci += 1
                    if ci == 2 and pending_tail is not None:
                        softmax_tail(*pending_tail)
                        pending_tail = None
                off += csz
            if b < B - 1:
                nc.scalar.activation(expv, sc[:S, :], AF.Exp)
                pending_tail = (b, sc, expv)
            else:
                softmax_tail(b, sc, expv, split=2, do_exp=True)
    nc.compile()
    return nc


def _get_bass():
    if "nc" not in _CACHE:
        _CACHE["nc"] = _build_bass()
    return _CACHE["nc"]


def kernel(hidden_state, encoder_outputs, W_att, b_att, W_v):
    from concourse.bass_utils import run_bass_kernel_spmd

    bf16 = ml_dtypes.bfloat16
    hidden_state = np.asarray(hidden_state, dtype=np.float32)
    W_att = np.asarray(W_att, dtype=np.float32)
    b_att = np.asarray(b_att, dtype=np.float32)
    W_v = np.asarray(W_v, dtype=np.float32)
    enc_bf = np.asarray(encoder_outputs, dtype=np.float32).astype(bf16)

    weT = np.ascontiguousarray(W_att[:, D:].T).astype(bf16)      # [E, K]
    whT = np.ascontiguousarray(W_att[:, :D].T).astype(bf16)      # [D, K]
    batt = np.ascontiguousarray(b_att.reshape(K, 1))
    wvs = np.zeros((K, 2 * S), dtype=np.float32)
    wvs[:, S - 1] = W_v[0]
    wvs = wvs.astype(bf16)

    in_maps = []
    for c in range(NCORES):
        h0 = c * HSH
        enc_c = np.ascontiguousarray(
            enc_bf[:, :, :, h0 : h0 + HSH, :]
        ).reshape(B, E, S * FREE)
        hid_c = np.ascontiguousarray(
            hidden_state[:, :, h0 : h0 + HSH, :]
        ).reshape(B, D, FREE).astype(bf16)
        in_maps.append(
            {"enc": enc_c, "hid": hid_c, "weT": weT, "whT": whT,
             "batt": batt, "wvs": wvs}
        )

    nc = _get_bass()
    kwargs = dict(_CACHE.get("run_kwargs", {}))
    res = run_bass_kernel_spmd(nc, in_maps, core_ids=list(range(NCORES)), **kwargs)
    _CACHE["last_result"] = res
    shards = [r["out"].reshape(B, S, HSH, W) for r in res.results]
    return np.concatenate(shards, axis=2)
